# revision 8
# baseline (speedup 1.0000x reference)
"""Trainium2 Bass kernel for sparse-conv (gather-GEMM) + BatchNorm + ReLU.

Contract: kernel(**inputs) takes the FULL unsharded inputs of
nn_BaseConvBlock (feats [1M,32] f32, W [27,32,32] f32, gamma/beta [32] f32,
nbr_idx [1M,27] i32, nbr_mask [1M,27] bool) and returns the full
[1M,32] f32 output, computed SPMD across 8 NeuronCores.

Sharding: voxel dim split 8 ways (125000/core, padded to 980 tiles of 128).
feats is replicated per core; W/gamma/beta replicated; BN batch stats
all-reduced on device ([128,2] f32 collective).

Per 128-voxel tile on each core:
  idxm = mask ? idx : 1e6 (DVE select; row 1e6 of the padded table is zero,
  and bounds_check skips those descriptors entirely)
  27x [P,1]-offset indirect DMA gathers -> G [128, 864] f32 (pre-zeroed)
  7x PE transpose -> GT; 7x accumulating matmul (W stationary, 4 offsets
  per 128-contraction block) -> PSUM y [32co, 128vox] at partition 32*(t%4)
  copy into SBUF-resident y_sb [128, T/4*128] (never round-trips HBM)
Then per-channel sum/sumsq (DVE reduce + ACT Square accum), AllReduce,
scale/shift, one fused ACT relu(y*scale+shift), PE transpose back, store.
"""
import contextlib
import ctypes
import sys
import types

import numpy as np

import concourse.bass as bass
import concourse.bacc as bacc
import concourse.tile as tile
from concourse import mybir
from concourse.masks import make_identity

P = 128
K = 27
CIN = COUT = 32
NROWS = 1_000_000
EPS = 1e-5
JBLK = 7
GW = K * CIN
NCORES = 8
T_TILES = 980            # 980*128 = 125440 >= 125000 per core
dt = mybir.dt
Alu = mybir.AluOpType
Act = mybir.ActivationFunctionType


def _build(n_tiles, n_total, idx_chunk=24):
    nc = bacc.Bacc("TRN2", num_devices=NCORES)
    feats = nc.declare_dram_parameter(
        "feats", [NROWS + 8, CIN], dt.float32, isOutput=False)
    idx_d = nc.declare_dram_parameter(
        "idx_sh", [P, n_tiles, K], dt.int32, isOutput=False)
    w_d = nc.declare_dram_parameter(
        "w_stack", [JBLK, P, COUT], dt.float32, isOutput=False)
    gam_d = nc.declare_dram_parameter(
        "gamma_r", [P, 1], dt.float32, isOutput=False)
    bet_d = nc.declare_dram_parameter(
        "beta_r", [P, 1], dt.float32, isOutput=False)
    fold_d = nc.declare_dram_parameter(
        "fold", [P, COUT], dt.float32, isOutput=False)
    out_d = nc.declare_dram_parameter(
        "out_sh", [P, n_tiles * COUT], dt.float32, isOutput=True)

    nq = (n_tiles + 3) // 4
    ycols = nq * P

    with tile.TileContext(nc) as tc:
        with tc.tile_pool(name="const", bufs=1) as cpool, \
             tc.tile_pool(name="ysb", bufs=1) as ypool, \
             tc.tile_pool(name="ix", bufs=3) as ipool, \
             tc.tile_pool(name="g", bufs=6) as gpool, \
             tc.tile_pool(name="gt", bufs=3) as gtpool, \
             tc.tile_pool(name="gtp", bufs=4, space="PSUM") as gtppool, \
             tc.tile_pool(name="yp", bufs=2, space="PSUM") as yppool, \
             tc.tile_pool(name="op", bufs=2, space="PSUM") as oppool, \
             tc.tile_pool(name="st", bufs=2) as stpool, \
             tc.tile_pool(name="dram", bufs=1, space="DRAM") as dpool:

            ident = cpool.tile([P, P], dt.float32)
            make_identity(nc, ident[:])
            wsb = cpool.tile([P, JBLK * COUT], dt.float32)
            nc.sync.dma_start(
                out=wsb[:].rearrange("p (j c) -> p j c", j=JBLK),
                in_=w_d[:].rearrange("j p c -> p j c"))
            gam = cpool.tile([P, 1], dt.float32)
            nc.sync.dma_start(out=gam[:], in_=gam_d[:])
            bet = cpool.tile([P, 1], dt.float32)
            nc.sync.dma_start(out=bet[:], in_=bet_d[:])

            y_sb = ypool.tile([P, ycols], dt.float32)

            # ---- pass 1: gather + conv ----
            for c0 in range(0, n_tiles, idx_chunk):
                c1 = min(c0 + idx_chunk, n_tiles)
                cw = (c1 - c0) * K
                im = ipool.tile([P, idx_chunk * K], dt.int32, tag="im")
                nc.sync.dma_start(
                    out=im[:, :cw],
                    in_=idx_d[:, c0:c1, :].rearrange("p t k -> p (t k)"))

                for t in range(c0, c1):
                    tl = t - c0
                    g = gpool.tile([P, GW], dt.float32, tag="g")
                    for k in range(K):
                        nc.gpsimd.indirect_dma_start(
                            out=g[:, k * CIN:(k + 1) * CIN],
                            out_offset=None,
                            in_=feats[:],
                            in_offset=bass.IndirectOffsetOnAxis(
                                ap=im[:, tl * K + k:tl * K + k + 1], axis=0),
                            bounds_check=NROWS,
                            oob_is_err=False,
                        )
                    gt = gtpool.tile([P, JBLK * P], dt.float32, tag="gt")
                    for j in range(JBLK):
                        kw = P if j < JBLK - 1 else GW - P * (JBLK - 1)
                        gp = gtppool.tile([P, P], dt.float32, tag="gp")
                        nc.tensor.transpose(
                            out=gp[:kw, :], in_=g[:, j * P:j * P + kw],
                            identity=ident[:])
                        nc.scalar.copy(
                            out=gt[:kw, j * P:(j + 1) * P], in_=gp[:kw, :])
                    q = t % 4
                    yp = yppool.tile([P, P], dt.float32, tag="yp")
                    for j in range(JBLK):
                        kw = P if j < JBLK - 1 else GW - P * (JBLK - 1)
                        nc.tensor.matmul(
                            out=yp[COUT * q:COUT * (q + 1), :],
                            lhsT=wsb[:kw, j * COUT:(j + 1) * COUT],
                            rhs=gt[:kw, j * P:(j + 1) * P],
                            start=(j == 0), stop=(j == JBLK - 1),
                            tile_position=(0, COUT * q),
                        )
                    nc.vector.tensor_copy(
                        out=y_sb[COUT * q:COUT * (q + 1),
                                 (t // 4) * P:(t // 4 + 1) * P],
                        in_=yp[COUT * q:COUT * (q + 1), :])

            # ---- BN stats ----
            SC = 512
            nchunk = (ycols + SC - 1) // SC
            s1p = cpool.tile([P, nchunk], dt.float32)
            s2p = cpool.tile([P, nchunk], dt.float32)
            scr = cpool.tile([P, SC], dt.float32)
            for ci, c0 in enumerate(range(0, ycols, SC)):
                c1 = min(c0 + SC, ycols)
                nc.vector.tensor_reduce(
                    out=s1p[:, ci:ci + 1], in_=y_sb[:, c0:c1],
                    axis=mybir.AxisListType.X, op=Alu.add)
                nc.scalar.activation(
                    out=scr[:, :c1 - c0], in_=y_sb[:, c0:c1],
                    func=Act.Square, accum_out=s2p[:, ci:ci + 1])
            s12 = cpool.tile([P, 2], dt.float32)
            nc.vector.tensor_reduce(
                out=s12[:, 0:1], in_=s1p[:], axis=mybir.AxisListType.X,
                op=Alu.add)
            nc.vector.tensor_reduce(
                out=s12[:, 1:2], in_=s2p[:], axis=mybir.AxisListType.X,
                op=Alu.add)

            cc_in = dpool.tile([P, 2], dt.float32)
            cc_out = dpool.tile([P, 2], dt.float32)
            nc.sync.dma_start(out=cc_in[:], in_=s12[:])
            nc.gpsimd.collective_compute(
                "AllReduce", Alu.add,
                replica_groups=[list(range(NCORES))],
                ins=[cc_in.opt()], outs=[cc_out.opt()])
            s12r = cpool.tile([P, 2], dt.float32)
            nc.sync.dma_start(out=s12r[:], in_=cc_out[:])

            fold = cpool.tile([P, COUT], dt.float32)
            nc.sync.dma_start(out=fold[:], in_=fold_d[:])
            sfold = oppool.tile([COUT, 2], dt.float32, tag="op")
            nc.tensor.matmul(out=sfold[:], lhsT=fold[:], rhs=s12r[:],
                             start=True, stop=True)
            mv = cpool.tile([COUT, 2], dt.float32)
            nc.vector.tensor_scalar_mul(mv[:], sfold[:], 1.0 / n_total)
            mean2 = cpool.tile([COUT, 1], dt.float32)
            nc.vector.tensor_tensor(
                out=mean2[:], in0=mv[:, 0:1], in1=mv[:, 0:1], op=Alu.mult)
            var = cpool.tile([COUT, 1], dt.float32)
            nc.vector.tensor_tensor(
                out=var[:], in0=mv[:, 1:2], in1=mean2[:], op=Alu.subtract)
            eps_t = cpool.tile([COUT, 1], dt.float32)
            nc.vector.memset(eps_t[:], EPS)
            std = cpool.tile([COUT, 1], dt.float32)
            nc.scalar.activation(out=std[:], in_=var[:], func=Act.Sqrt,
                                 bias=eps_t[:])
            rstd = cpool.tile([COUT, 1], dt.float32)
            nc.vector.reciprocal(out=rstd[:], in_=std[:])
            pk = cpool.tile([COUT, 2], dt.float32)
            nc.vector.tensor_copy(out=pk[:, 0:1], in_=mv[:, 0:1])
            nc.vector.tensor_copy(out=pk[:, 1:2], in_=rstd[:])
            mr_d = dpool.tile([COUT, 2], dt.float32)
            nc.sync.dma_start(out=mr_d[:], in_=pk[:])
            mr = cpool.tile([P, 2], dt.float32)
            for q in range(4):
                nc.sync.dma_start(
                    out=mr[COUT * q:COUT * (q + 1), :], in_=mr_d[:])
            scale = cpool.tile([P, 1], dt.float32)
            nc.vector.tensor_tensor(
                out=scale[:], in0=gam[:], in1=mr[:, 1:2], op=Alu.mult)
            shift = cpool.tile([P, 1], dt.float32)
            nc.vector.tensor_tensor(
                out=shift[:], in0=mr[:, 0:1], in1=scale[:], op=Alu.mult)
            nc.vector.tensor_tensor(
                out=shift[:], in0=bet[:], in1=shift[:], op=Alu.subtract)

            # ---- pass 2: normalize + relu + transpose + store ----
            nc.scalar.activation(
                out=y_sb[:], in_=y_sb[:], func=Act.Relu,
                scale=scale[:], bias=shift[:])
            ochunk = 32
            for c0 in range(0, n_tiles, ochunk):
                c1 = min(c0 + ochunk, n_tiles)
                st = stpool.tile([P, ochunk * COUT], dt.float32, tag="st")
                for t in range(c0, c1):
                    q = t % 4
                    op = oppool.tile([P, COUT], dt.float32, tag="op")
                    nc.tensor.transpose(
                        out=op[:],
                        in_=y_sb[COUT * q:COUT * (q + 1),
                                 (t // 4) * P:(t // 4 + 1) * P],
                        identity=ident[COUT * q:COUT * (q + 1),
                                       COUT * q:COUT * (q + 1)],
                        tile_position=(COUT * q, 0))
                    nc.scalar.copy(
                        out=st[:, (t - c0) * COUT:(t - c0 + 1) * COUT],
                        in_=op[:])
                nc.sync.dma_start(
                    out=out_d[:, c0 * COUT:c1 * COUT],
                    in_=st[:, :(c1 - c0) * COUT])
    return nc


def _install_ntff_hook():
    """The container's antenv lacks axon_hooks; provide it so trace=True
    works (harmless if never used)."""
    if "antenv.axon_hooks" in sys.modules:
        return
    try:
        lib = ctypes.CDLL("/opt/axon/libaxon_pjrt.so")
        lib.axon_start_nrt_profile.argtypes = [
            ctypes.POINTER(ctypes.c_int64), ctypes.c_size_t]
        lib.axon_start_nrt_profile.restype = ctypes.c_int64
        lib.axon_stop_nrt_profile.argtypes = [ctypes.c_char_p]
        lib.axon_stop_nrt_profile.restype = ctypes.c_int64
    except OSError:
        return

    @contextlib.contextmanager
    def _hook(output_dir, device_ids):
        import jax
        jax.devices()
        if device_ids:
            ids = (ctypes.c_int64 * len(device_ids))(*device_ids)
            rc = lib.axon_start_nrt_profile(ids, len(device_ids))
        else:
            rc = lib.axon_start_nrt_profile(None, 0)
        if rc != 0:
            raise RuntimeError(f"axon_start_nrt_profile rc={rc}")
        try:
            yield
        finally:
            n = lib.axon_stop_nrt_profile(str(output_dir).encode())
            if n <= 0:
                print(f"profile: {n} files in {output_dir}", file=sys.stderr)

    mod = types.ModuleType("antenv.axon_hooks")
    mod.get_axon_ntff_profile_hook = lambda: _hook
    mod.set_axon_ntff_profile_hook = lambda h: None
    sys.modules["antenv.axon_hooks"] = mod


_NC_CACHE = {}


def _get_nc():
    if "nc" not in _NC_CACHE:
        _NC_CACHE["nc"] = _build(T_TILES, NROWS)
        _NC_CACHE["nc"].finalize()
    return _NC_CACHE["nc"]


def kernel(feats, W, gamma, beta, nbr_idx, nbr_mask, trace=False):
    feats = np.asarray(feats, np.float32)
    W = np.asarray(W, np.float32)
    gamma = np.asarray(gamma, np.float32)
    beta = np.asarray(beta, np.float32)
    nbr_idx = np.asarray(nbr_idx, np.int32)
    nbr_mask = np.asarray(nbr_mask, bool)
    n = feats.shape[0]
    assert n == NROWS and n % NCORES == 0

    # host layout prep (data-independent)
    feats_pad = np.zeros((NROWS + 8, CIN), np.float32)
    feats_pad[:n] = feats
    w_stack = np.zeros((JBLK, P, COUT), np.float32)
    for k in range(K):
        j, m = k // 4, k % 4
        w_stack[j, 32 * m:32 * (m + 1), :] = W[k]
    gamma_r = np.tile(gamma.reshape(COUT, 1), (4, 1))
    beta_r = np.tile(beta.reshape(COUT, 1), (4, 1))
    fold = np.tile(np.eye(COUT, dtype=np.float32), (4, 1))

    # sentinel-encode: masked -> NROWS (zero row of feats_pad, real transfer)
    im_full = np.where(nbr_mask, nbr_idx, np.int32(NROWS)).astype(np.int32)

    per = T_TILES * P
    in_maps = []
    for c in range(NCORES):
        lo, hi = c * (n // NCORES), (c + 1) * (n // NCORES)
        cnt = hi - lo
        idx = np.full((per, K), NROWS, np.int32)
        idx[:cnt] = im_full[lo:hi]
        in_maps.append(dict(
            feats=feats_pad,
            idx_sh=np.ascontiguousarray(
                idx.reshape(T_TILES, P, K).transpose(1, 0, 2)),
            w_stack=w_stack, gamma_r=gamma_r, beta_r=beta_r, fold=fold))

    _install_ntff_hook()
    from concourse import bass_utils
    bass_utils.upload_artifacts = lambda tmpdir: tmpdir
    nc = _get_nc()
    res = bass_utils.run_bass_kernel_spmd(
        nc, in_maps, core_ids=list(range(NCORES)), trace=trace)

    chunks = []
    for c in range(NCORES):
        o = res.results[c]["out_sh"].reshape(P, T_TILES, COUT)
        o = o.transpose(1, 0, 2).reshape(per, COUT)
        chunks.append(o[:n // NCORES])
    out = np.concatenate(chunks, axis=0)
    if trace:
        kernel.last_exec_time_ns = res.exec_time_ns
        kernel.last_trace = (res.instructions_and_trace or (None, None))[1]
    return out



# revision 11
# speedup vs baseline: 1.1003x; 1.1003x over previous
"""Trainium2 Bass kernel for sparse-conv + BN + ReLU, Z-table formulation.

Key idea: the per-(voxel,offset) contribution feats[nbr[i,k]] @ W[k] is a
row of the precomputed table Z[j*27+k] = feats[j] @ W[k] (bf16, built on
device: 7813 PE matmuls + 1.7 GB HBM writes). Gathering from Z makes
descriptors k-agnostic, so the ~50% masked pairs can be PACKED OUT on the
host: voxels are sorted by valid-neighbor count and dealt into 128-voxel
tiles whose slot budget C_t is the max count in the tile (~13.4k gather
instructions/core total vs 26,460 for the direct formulation — the Pool
engine's ~1.16us/indirect-DMA is the bottleneck). Each gather instruction
fetches 128 rows of Z into one 32-column slot block; a strided-view DVE
reduce sums the slots into the SBUF-resident y tile. Padding slots read a
zero row of Z.

y layout is voxel-on-partition [128, 980*32], so BN stats use a
strided-view DVE reduce + ones-vector PE matmul for the partition axis,
the per-channel scale/shift is broadcast via a PE outer product, applied
with DVE, and the store is a direct (transpose-free) DMA. The host
un-permutes the sorted voxel order afterwards.
"""
import contextlib
import ctypes
import os
import sys
import types

os.environ["NEURON_SCRATCHPAD_PAGE_SIZE"] = "2048"

import ml_dtypes
import numpy as np

import concourse.bass as bass
import concourse.bacc as bacc
import concourse.tile as tile
from concourse import mybir
from concourse.masks import make_identity

P = 128
K = 27
CIN = COUT = 32
NROWS = 1_000_000
EPS = 1e-5
NCORES = 8
T_TILES = 980              # 980*128 = 125440 >= 125000 per core
NJT = 7813                 # ceil((NROWS+63)/128): j-tiles for Z build
NJROWS = NJT * P           # 1000064 feats rows incl. zero pad
ZROWS = NJROWS * K         # 27001728 Z rows
ZROW = NROWS * K           # 27000000: zero row (j=1M is zero-padded)
FCH = 32                   # j-tiles per feats load chunk
ZB = 4                     # j-tiles per Z writeback
dt = mybir.dt
Alu = mybir.AluOpType
Act = mybir.ActivationFunctionType


def _build(schedule, n_total):
    n_tiles = len(schedule)
    totc = int(sum(schedule))
    nc = bacc.Bacc("TRN2", num_devices=NCORES)
    feats_b = nc.declare_dram_parameter(
        "feats_b", [NJROWS, CIN], dt.bfloat16, isOutput=False)
    wall_d = nc.declare_dram_parameter(
        "wall4", [P, K * COUT], dt.bfloat16, isOutput=False)
    pk_d = nc.declare_dram_parameter(
        "pk", [P, totc], dt.int32, isOutput=False)
    gam_d = nc.declare_dram_parameter(
        "gamma_r", [1, COUT], dt.float32, isOutput=False)
    bet_d = nc.declare_dram_parameter(
        "beta_r", [1, COUT], dt.float32, isOutput=False)
    ones_d = nc.declare_dram_parameter(
        "ones_r", [1, P], dt.float32, isOutput=False)
    out_d = nc.declare_dram_parameter(
        "out_sh", [P, n_tiles * COUT], dt.float32, isOutput=True)

    ycols = n_tiles * COUT

    with tile.TileContext(nc) as tc:
        with tc.tile_pool(name="const", bufs=1) as cpool, \
             tc.tile_pool(name="ysb", bufs=1) as ypool, \
             tc.tile_pool(name="ft", bufs=3) as fpool, \
             tc.tile_pool(name="tr", bufs=3) as trpool, \
             tc.tile_pool(name="zw", bufs=3) as zwpool, \
             tc.tile_pool(name="pkp", bufs=3) as pkpool, \
             tc.tile_pool(name="ga", bufs=4) as apool, \
             tc.tile_pool(name="trp", bufs=2, space="PSUM") as trppool, \
             tc.tile_pool(name="zp", bufs=4, space="PSUM") as zppool, \
             tc.tile_pool(name="sp", bufs=2, space="PSUM") as sppool, \
             tc.tile_pool(name="st", bufs=2) as stpool, \
             tc.tile_pool(name="zd", bufs=1, space="DRAM") as zdpool, \
             tc.tile_pool(name="dram", bufs=1, space="DRAM") as dpool:

            identb = cpool.tile([P, P], dt.bfloat16)
            make_identity(nc, identb[:])
            wall4 = cpool.tile([P, K * COUT], dt.bfloat16)
            nc.sync.dma_start(out=wall4[:], in_=wall_d[:])
            gam = cpool.tile([1, COUT], dt.float32)
            nc.sync.dma_start(out=gam[:], in_=gam_d[:])
            bet = cpool.tile([1, COUT], dt.float32)
            nc.sync.dma_start(out=bet[:], in_=bet_d[:])
            ones_r = cpool.tile([1, P], dt.float32)
            nc.sync.dma_start(out=ones_r[:], in_=ones_d[:])
            ones_c = cpool.tile([P, 1], dt.float32)
            nc.vector.memset(ones_c[:], 1.0)

            zt = zdpool.tile([ZROWS, CIN], dt.bfloat16)

            # ---- phase A: Z = feats @ W_all, written j-tile-wise ----
            for ch0 in range(0, NJT, FCH):
                ch1 = min(ch0 + FCH, NJT)
                nt = ch1 - ch0
                ft = fpool.tile([P, FCH * CIN], dt.bfloat16, tag="ft")
                nc.sync.dma_start(
                    out=ft[:, :nt * CIN].rearrange(
                        "p (t c) -> p t c", c=CIN),
                    in_=feats_b[ch0 * P:ch1 * P, :].rearrange(
                        "(t p) c -> p t c", p=P))
                for b0 in range(0, nt, 4):
                    b1 = min(b0 + 4, nt)
                    nb = b1 - b0
                    kw = nb * CIN
                    trp = trppool.tile([P, P], dt.bfloat16, tag="trp")
                    nc.tensor.transpose(
                        out=trp[:kw, :],
                        in_=ft[:, b0 * CIN:b0 * CIN + kw],
                        identity=identb[:])
                    trs = trpool.tile([P, P], dt.bfloat16, tag="tr")
                    nc.scalar.copy(out=trs[:kw, :], in_=trp[:kw, :])
                    zw = zwpool.tile([P, ZB * K * COUT], dt.bfloat16,
                                     tag="zw")
                    for b in range(b0, b1):
                        bb = b - b0
                        for h in range(2):
                            hw = K * COUT // 2
                            zp = zppool.tile([P, hw], dt.float32, tag="zp")
                            nc.tensor.matmul(
                                out=zp[:],
                                lhsT=trs[CIN * bb:CIN * (bb + 1), :],
                                rhs=wall4[CIN * bb:CIN * (bb + 1),
                                          h * hw:(h + 1) * hw],
                                start=True, stop=True,
                                tile_position=(CIN * bb, 0),
                            )
                            zwo = zw[:, bb * K * COUT + h * hw:
                                     bb * K * COUT + (h + 1) * hw]
                            if h == 0:
                                nc.scalar.copy(out=zwo, in_=zp[:])
                            else:
                                nc.vector.tensor_copy(out=zwo, in_=zp[:])
                    nc.sync.dma_start(
                        out=zt[(ch0 + b0) * P * K:(ch0 + b1) * P * K, :]
                        .rearrange("(b p k) c -> p b k c", b=nb, p=P),
                        in_=zw[:, :nb * K * COUT].rearrange(
                            "p (b k c) -> p b k c", b=nb, c=CIN))

            # ---- phase B: packed gather-sum from Z ----
            y_sb = ypool.tile([P, ycols], dt.float32)
            nc.vector.memset(y_sb[:], 0)

            # chunk pk loads on tile boundaries, ~512 cols per load
            chunks = []
            cur = []
            cw = 0
            for t in range(n_tiles):
                if cw + schedule[t] > 544 and cur:
                    chunks.append(cur)
                    cur, cw = [], 0
                cur.append(t)
                cw += schedule[t]
            if cur:
                chunks.append(cur)

            cb = 0
            for tl_list in chunks:
                ccols = int(sum(schedule[t] for t in tl_list))
                if ccols == 0:
                    continue
                pk = pkpool.tile([P, 576], dt.int32, tag="pk")
                nc.sync.dma_start(
                    out=pk[:, :ccols], in_=pk_d[:, cb:cb + ccols])
                lc = 0
                for t in tl_list:
                    ct = int(schedule[t])
                    if ct == 0:
                        continue
                    ga = apool.tile([P, K * COUT], dt.bfloat16, tag="ga")
                    for c in range(ct):
                        nc.gpsimd.indirect_dma_start(
                            out=ga[:, c * COUT:(c + 1) * COUT],
                            out_offset=None,
                            in_=zt[:],
                            in_offset=bass.IndirectOffsetOnAxis(
                                ap=pk[:, lc + c:lc + c + 1], axis=0),
                            bounds_check=ZROWS - 1,
                            oob_is_err=False,
                        )
                    nc.vector.tensor_reduce(
                        out=y_sb[:, t * COUT:(t + 1) * COUT],
                        in_=ga[:, :ct * COUT].rearrange(
                            "p (n c) -> p c n", c=COUT),
                        axis=mybir.AxisListType.X, op=Alu.add)
                    lc += ct
                cb += ccols

            # ---- BN stats ----
            r1 = cpool.tile([P, COUT], dt.float32)
            nc.vector.tensor_reduce(
                out=r1[:], in_=y_sb[:].rearrange("p (n c) -> p c n", c=COUT),
                axis=mybir.AxisListType.X, op=Alu.add)
            r2 = cpool.tile([P, COUT], dt.float32)
            scr = cpool.tile([P, 512], dt.float32)
            r2c = cpool.tile([P, COUT], dt.float32)
            for ci, c0 in enumerate(range(0, ycols, 512)):
                c1 = min(c0 + 512, ycols)
                nc.scalar.activation(
                    out=scr[:, :c1 - c0], in_=y_sb[:, c0:c1],
                    func=Act.Square)
                nc.vector.tensor_reduce(
                    out=(r2 if ci == 0 else r2c)[:],
                    in_=scr[:, :c1 - c0].rearrange(
                        "p (n c) -> p c n", c=COUT),
                    axis=mybir.AxisListType.X, op=Alu.add)
                if ci > 0:
                    nc.vector.tensor_tensor(
                        out=r2[:], in0=r2[:], in1=r2c[:], op=Alu.add)
            r12 = cpool.tile([P, 2 * COUT], dt.float32)
            nc.vector.tensor_copy(out=r12[:, :COUT], in_=r1[:])
            nc.vector.tensor_copy(out=r12[:, COUT:], in_=r2[:])
            sp = sppool.tile([1, 2 * COUT], dt.float32, tag="sp")
            nc.tensor.matmul(out=sp[:], lhsT=ones_c[:], rhs=r12[:],
                             start=True, stop=True)
            s12 = cpool.tile([1, 2 * COUT], dt.float32)
            nc.scalar.copy(out=s12[:], in_=sp[:])

            cc_in = dpool.tile([1, 2 * COUT], dt.float32)
            cc_out = dpool.tile([1, 2 * COUT], dt.float32)
            nc.sync.dma_start(out=cc_in[:], in_=s12[:])
            nc.gpsimd.collective_compute(
                "AllReduce", Alu.add,
                replica_groups=[list(range(NCORES))],
                ins=[cc_in.opt()], outs=[cc_out.opt()])
            s12r = cpool.tile([1, 2 * COUT], dt.float32)
            nc.sync.dma_start(out=s12r[:], in_=cc_out[:])

            mv = cpool.tile([1, 2 * COUT], dt.float32)
            nc.vector.tensor_scalar_mul(mv[:], s12r[:], 1.0 / n_total)
            mean2 = cpool.tile([1, COUT], dt.float32)
            nc.vector.tensor_tensor(
                out=mean2[:], in0=mv[:, :COUT], in1=mv[:, :COUT],
                op=Alu.mult)
            var = cpool.tile([1, COUT], dt.float32)
            nc.vector.tensor_tensor(
                out=var[:], in0=mv[:, COUT:], in1=mean2[:], op=Alu.subtract)
            nc.vector.tensor_scalar_add(var[:], var[:], EPS)
            std = cpool.tile([1, COUT], dt.float32)
            nc.scalar.activation(out=std[:], in_=var[:], func=Act.Sqrt)
            rstd = cpool.tile([1, COUT], dt.float32)
            nc.vector.reciprocal(out=rstd[:], in_=std[:])
            sc_row = cpool.tile([1, COUT], dt.float32)
            nc.vector.tensor_tensor(
                out=sc_row[:], in0=gam[:], in1=rstd[:], op=Alu.mult)
            sh_row = cpool.tile([1, COUT], dt.float32)
            nc.vector.tensor_tensor(
                out=sh_row[:], in0=mv[:, :COUT], in1=sc_row[:], op=Alu.mult)
            nc.vector.tensor_tensor(
                out=sh_row[:], in0=bet[:], in1=sh_row[:], op=Alu.subtract)

            # broadcast rows -> [P, 32] via outer product, then tile to 512
            ssp = sppool.tile([P, 2 * COUT], dt.float32, tag="sp")
            nc.tensor.matmul(out=ssp[:, :COUT], lhsT=ones_r[:],
                             rhs=sc_row[:], start=True, stop=True)
            nc.tensor.matmul(out=ssp[:, COUT:], lhsT=ones_r[:],
                             rhs=sh_row[:], start=True, stop=True)
            sc_rep = cpool.tile([P, 512], dt.float32)
            sh_rep = cpool.tile([P, 512], dt.float32)
            for r in range(512 // COUT):
                nc.scalar.copy(
                    out=sc_rep[:, r * COUT:(r + 1) * COUT], in_=ssp[:, :COUT])
                nc.scalar.copy(
                    out=sh_rep[:, r * COUT:(r + 1) * COUT], in_=ssp[:, COUT:])

            # ---- pass 2: y = relu(y*scale + shift), store ----
            for c0 in range(0, ycols, 512):
                c1 = min(c0 + 512, ycols)
                w = c1 - c0
                nc.vector.tensor_tensor(
                    out=y_sb[:, c0:c1], in0=y_sb[:, c0:c1],
                    in1=sc_rep[:, :w], op=Alu.mult)
                nc.vector.tensor_tensor(
                    out=y_sb[:, c0:c1], in0=y_sb[:, c0:c1],
                    in1=sh_rep[:, :w], op=Alu.add)
                nc.vector.tensor_scalar_max(y_sb[:, c0:c1], y_sb[:, c0:c1],
                                            0.0)
            for c0 in range(0, ycols, 4096):
                c1 = min(c0 + 4096, ycols)
                nc.sync.dma_start(out=out_d[:, c0:c1], in_=y_sb[:, c0:c1])
    return nc


def _install_ntff_hook():
    if "antenv.axon_hooks" in sys.modules:
        return
    try:
        lib = ctypes.CDLL("/opt/axon/libaxon_pjrt.so")
        lib.axon_start_nrt_profile.argtypes = [
            ctypes.POINTER(ctypes.c_int64), ctypes.c_size_t]
        lib.axon_start_nrt_profile.restype = ctypes.c_int64
        lib.axon_stop_nrt_profile.argtypes = [ctypes.c_char_p]
        lib.axon_stop_nrt_profile.restype = ctypes.c_int64
    except OSError:
        return

    @contextlib.contextmanager
    def _hook(output_dir, device_ids):
        import jax
        jax.devices()
        if device_ids:
            ids = (ctypes.c_int64 * len(device_ids))(*device_ids)
            rc = lib.axon_start_nrt_profile(ids, len(device_ids))
        else:
            rc = lib.axon_start_nrt_profile(None, 0)
        if rc != 0:
            raise RuntimeError(f"axon_start_nrt_profile rc={rc}")
        try:
            yield
        finally:
            n = lib.axon_stop_nrt_profile(str(output_dir).encode())
            if n <= 0:
                print(f"profile: {n} files in {output_dir}", file=sys.stderr)

    mod = types.ModuleType("antenv.axon_hooks")
    mod.get_axon_ntff_profile_hook = lambda: _hook
    mod.set_axon_ntff_profile_hook = lambda h: None
    sys.modules["antenv.axon_hooks"] = mod


_NC_CACHE = {}


def _get_nc(schedule):
    key = tuple(schedule)
    if key not in _NC_CACHE:
        nc = _build(schedule, NROWS)
        nc.finalize()
        _NC_CACHE[key] = nc
    return _NC_CACHE[key]


def kernel(feats, W, gamma, beta, nbr_idx, nbr_mask, trace=False):
    feats = np.asarray(feats, np.float32)
    W = np.asarray(W, np.float32)
    gamma = np.asarray(gamma, np.float32)
    beta = np.asarray(beta, np.float32)
    nbr_idx = np.asarray(nbr_idx, np.int32)
    nbr_mask = np.asarray(nbr_mask, bool)
    n = feats.shape[0]
    assert n == NROWS and n % NCORES == 0
    per_core = n // NCORES

    feats_b = np.zeros((NJROWS, CIN), ml_dtypes.bfloat16)
    feats_b[:n] = feats.astype(ml_dtypes.bfloat16)
    # wall4[32a + ci, k*32 + co] = W[k][ci, co] replicated on 4 bands
    wall = np.ascontiguousarray(
        W.transpose(1, 0, 2).reshape(CIN, K * COUT))
    wall4 = np.tile(wall, (4, 1)).astype(ml_dtypes.bfloat16)
    gamma_r = gamma.reshape(1, COUT)
    beta_r = beta.reshape(1, COUT)
    ones_r = np.ones((1, P), np.float32)

    # packed jk indices: valid neighbors first, ZROW padding
    jk = nbr_idx * np.int32(K) + np.arange(K, dtype=np.int32)[None, :]
    ordk = np.argsort(~nbr_mask, axis=1, kind="stable")
    jkp = np.take_along_axis(jk, ordk, 1)
    mskp = np.take_along_axis(nbr_mask, ordk, 1)
    jkp = np.where(mskp, jkp, np.int32(ZROW))
    counts = nbr_mask.sum(1).astype(np.int32)

    orders = []
    cts = np.zeros((NCORES, T_TILES), np.int32)
    for c in range(NCORES):
        lo = c * per_core
        cs = counts[lo:lo + per_core]
        order = np.argsort(-cs, kind="stable")
        orders.append(order)
        sc = np.concatenate(
            [cs[order], np.zeros(T_TILES * P - per_core, np.int32)])
        cts[c] = sc[::P][:T_TILES]
    schedule = cts.max(0)
    totc = int(schedule.sum())
    col_base = np.concatenate([[0], np.cumsum(schedule)[:-1]])

    in_maps = []
    for c in range(NCORES):
        lo = c * per_core
        R = np.full((T_TILES * P, K), ZROW, np.int32)
        R[:per_core] = jkp[lo:lo + per_core][orders[c]]
        R3 = R.reshape(T_TILES, P, K)
        pk = np.full((P, totc), ZROW, np.int32)
        for t in range(T_TILES):
            s = int(schedule[t])
            if s:
                pk[:, col_base[t]:col_base[t] + s] = R3[t][:, :s]
        in_maps.append(dict(
            feats_b=feats_b, wall4=wall4, pk=pk,
            gamma_r=gamma_r, beta_r=beta_r, ones_r=ones_r))

    _install_ntff_hook()
    from concourse import bass_utils
    bass_utils.upload_artifacts = lambda tmpdir: tmpdir
    nc = _get_nc(schedule.tolist())
    res = bass_utils.run_bass_kernel_spmd(
        nc, in_maps, core_ids=list(range(NCORES)), trace=trace)

    chunks = []
    for c in range(NCORES):
        o = res.results[c]["out_sh"].reshape(P, T_TILES, COUT)
        ys = o.transpose(1, 0, 2).reshape(T_TILES * P, COUT)
        rc = np.empty((per_core, COUT), np.float32)
        rc[orders[c]] = ys[:per_core]
        chunks.append(rc)
    out = np.concatenate(chunks, axis=0)
    if trace:
        kernel.last_exec_time_ns = res.exec_time_ns
        kernel.last_trace = (res.instructions_and_trace or (None, None))[1]
    return out


# revision 18
# speedup vs baseline: 1.1175x; 1.0157x over previous
"""Trainium2 Bass kernel for sparse-conv + BN + ReLU, Z-table formulation.

Key idea: the per-(voxel,offset) contribution feats[nbr[i,k]] @ W[k] is a
row of the precomputed table Z[j*27+k] = feats[j] @ W[k] (bf16, built on
device: 7813 PE matmuls + 1.7 GB HBM writes). Gathering from Z makes
descriptors k-agnostic, so the ~50% masked pairs can be PACKED OUT on the
host: voxels are sorted by valid-neighbor count and dealt into 128-voxel
tiles whose slot budget C_t is the max count in the tile (~13.4k gather
instructions/core total vs 26,460 for the direct formulation — the Pool
engine's ~1.16us/indirect-DMA is the bottleneck). Each gather instruction
fetches 128 rows of Z into one 32-column slot block; a strided-view DVE
reduce sums the slots into the SBUF-resident y tile. Padding slots read a
zero row of Z.

y layout is voxel-on-partition [128, 980*32], so BN stats use a
strided-view DVE reduce + ones-vector PE matmul for the partition axis,
the per-channel scale/shift is broadcast via a PE outer product, applied
with DVE, and the store is a direct (transpose-free) DMA. The host
un-permutes the sorted voxel order afterwards.
"""
import contextlib
import ctypes
import os
import sys
import types

os.environ["NEURON_SCRATCHPAD_PAGE_SIZE"] = "2048"

import ml_dtypes
import numpy as np

import concourse.bass as bass
import concourse.bacc as bacc
import concourse.tile as tile
from concourse import mybir
from concourse.masks import make_identity

P = 128
K = 27
CIN = COUT = 32
NROWS = 1_000_000
EPS = 1e-5
NCORES = 8
T_TILES = 980              # 980*128 = 125440 >= 125000 per core
NJT = 7813                 # ceil((NROWS+63)/128): j-tiles for Z build
NJROWS = NJT * P           # 1000064 feats rows incl. zero pad
ZROWS = NJROWS * K         # 27001728 Z rows
ZROW = NROWS * K           # 27000000: zero row (j=1M is zero-padded)
FCH = 32                   # j-tiles per feats load chunk
ZB = 4                     # j-tiles per Z writeback
dt = mybir.dt
Alu = mybir.AluOpType
Act = mybir.ActivationFunctionType


def _build(schedule, n_total):
    n_tiles = len(schedule)
    totc = int(sum(schedule))
    nc = bacc.Bacc("TRN2", num_devices=NCORES)
    feats_b = nc.declare_dram_parameter(
        "feats_b", [NJROWS, CIN], dt.bfloat16, isOutput=False)
    wall_d = nc.declare_dram_parameter(
        "wall8", [P, 2 * K * COUT], dt.bfloat16, isOutput=False)
    pk_d = nc.declare_dram_parameter(
        "pk", [P, totc], dt.int32, isOutput=False)
    gam_d = nc.declare_dram_parameter(
        "gamma_r", [1, COUT], dt.float32, isOutput=False)
    bet_d = nc.declare_dram_parameter(
        "beta_r", [1, COUT], dt.float32, isOutput=False)
    ones_d = nc.declare_dram_parameter(
        "ones_r", [1, P], dt.float32, isOutput=False)
    out_d = nc.declare_dram_parameter(
        "out_sh", [P, n_tiles * COUT], dt.float32, isOutput=True)

    ycols = n_tiles * COUT

    with tile.TileContext(nc) as tc:
        with tc.tile_pool(name="const", bufs=1) as cpool, \
             tc.tile_pool(name="ysb", bufs=1) as ypool, \
             tc.tile_pool(name="ft", bufs=3) as fpool, \
             tc.tile_pool(name="tr", bufs=3) as trpool, \
             tc.tile_pool(name="zw", bufs=3) as zwpool, \
             tc.tile_pool(name="pkp", bufs=4) as pkpool, \
             tc.tile_pool(name="ga", bufs=6) as apool, \
             tc.tile_pool(name="trp", bufs=2, space="PSUM") as trppool, \
             tc.tile_pool(name="zp", bufs=4, space="PSUM") as zppool, \
             tc.tile_pool(name="sp", bufs=2, space="PSUM") as sppool, \
             tc.tile_pool(name="st", bufs=2) as stpool, \
             tc.tile_pool(name="zd", bufs=1, space="DRAM") as zdpool, \
             tc.tile_pool(name="dram", bufs=1, space="DRAM") as dpool:

            identb = cpool.tile([P, P], dt.bfloat16)
            make_identity(nc, identb[:])
            wall8 = cpool.tile([P, 2 * K * COUT], dt.bfloat16)
            nc.sync.dma_start(out=wall8[:], in_=wall_d[:])
            gam = cpool.tile([1, COUT], dt.float32)
            nc.sync.dma_start(out=gam[:], in_=gam_d[:])
            bet = cpool.tile([1, COUT], dt.float32)
            nc.sync.dma_start(out=bet[:], in_=bet_d[:])
            ones_r = cpool.tile([1, P], dt.float32)
            nc.sync.dma_start(out=ones_r[:], in_=ones_d[:])
            ones_c = cpool.tile([P, 1], dt.float32)
            nc.vector.memset(ones_c[:], 1.0)

            zt = zdpool.tile([ZROWS, CIN], dt.bfloat16)

            # ---- phase A: Z = feats @ W_all, written j-tile-wise ----
            for ch0 in range(0, NJT, FCH):
                ch1 = min(ch0 + FCH, NJT)
                nt = ch1 - ch0
                ft = fpool.tile([P, FCH * CIN], dt.bfloat16, tag="ft")
                nc.sync.dma_start(
                    out=ft[:, :nt * CIN].rearrange(
                        "p (t c) -> p t c", c=CIN),
                    in_=feats_b[ch0 * P:ch1 * P, :].rearrange(
                        "(t p) c -> p t c", p=P))
                for b0 in range(0, nt, 4):
                    b1 = min(b0 + 4, nt)
                    nb = b1 - b0
                    kw = nb * CIN
                    trp = trppool.tile([P, P], dt.bfloat16, tag="trp")
                    nc.tensor.transpose(
                        out=trp[:kw, :],
                        in_=ft[:, b0 * CIN:b0 * CIN + kw],
                        identity=identb[:])
                    trs = trpool.tile([P, P], dt.bfloat16, tag="tr")
                    nc.scalar.copy(out=trs[:kw, :], in_=trp[:kw, :])
                    zw = zwpool.tile([P, ZB * K * COUT], dt.bfloat16,
                                     tag="zw")
                    hw = K * COUT // 2
                    for pp in range(0, (nb + 1) // 2):
                        po = 2 * CIN * pp
                        if 2 * pp + 1 < nb:
                            # tile pair: 64-contraction block-diagonal
                            nq, cw2, lw = 4, 2 * CIN, 2 * K * COUT
                        else:
                            nq, cw2, lw = 2, CIN, K * COUT
                        for q in range(nq):
                            zp = zppool.tile([P, hw], dt.float32, tag="zp")
                            nc.tensor.matmul(
                                out=zp[:],
                                lhsT=trs[po:po + cw2, :],
                                rhs=wall8[po:po + cw2, q * hw:(q + 1) * hw],
                                start=True, stop=True,
                                tile_position=(po, 0),
                            )
                            zwo = zw[:, 2 * pp * K * COUT + q * hw:
                                     2 * pp * K * COUT + (q + 1) * hw]
                            if q % 2 == 0:
                                nc.scalar.copy(out=zwo, in_=zp[:])
                            else:
                                nc.vector.tensor_copy(out=zwo, in_=zp[:])
                    nc.sync.dma_start(
                        out=zt[(ch0 + b0) * P * K:(ch0 + b1) * P * K, :]
                        .rearrange("(b p k) c -> p b k c", b=nb, p=P),
                        in_=zw[:, :nb * K * COUT].rearrange(
                            "p (b k c) -> p b k c", b=nb, c=CIN))

            # ---- phase B: packed gather-sum from Z ----
            y_sb = ypool.tile([P, ycols], dt.float32)
            nc.vector.memset(y_sb[:], 0)

            # chunk pk loads on tile boundaries, ~512 cols per load
            chunks = []
            cur = []
            cw = 0
            for t in range(n_tiles):
                if cw + schedule[t] > 544 and cur:
                    chunks.append(cur)
                    cur, cw = [], 0
                cur.append(t)
                cw += schedule[t]
            if cur:
                chunks.append(cur)

            cb = 0
            for tl_list in chunks:
                ccols = int(sum(schedule[t] for t in tl_list))
                if ccols == 0:
                    continue
                pk = pkpool.tile([P, 576], dt.int32, tag="pk")
                nc.sync.dma_start(
                    out=pk[:, :ccols], in_=pk_d[:, cb:cb + ccols])
                lc = 0
                for t in tl_list:
                    ct = int(schedule[t])
                    if ct == 0:
                        continue
                    ga = apool.tile([P, K * COUT], dt.bfloat16, tag="ga")
                    for c in range(ct):
                        nc.gpsimd.indirect_dma_start(
                            out=ga[:, c * COUT:(c + 1) * COUT],
                            out_offset=None,
                            in_=zt[:],
                            in_offset=bass.IndirectOffsetOnAxis(
                                ap=pk[:, lc + c:lc + c + 1], axis=0),
                        )
                    nc.vector.tensor_reduce(
                        out=y_sb[:, t * COUT:(t + 1) * COUT],
                        in_=ga[:, :ct * COUT].rearrange(
                            "p (n c) -> p c n", c=COUT),
                        axis=mybir.AxisListType.X, op=Alu.add)
                    lc += ct
                cb += ccols

            # ---- BN stats ----
            r1 = cpool.tile([P, COUT], dt.float32)
            nc.vector.tensor_reduce(
                out=r1[:], in_=y_sb[:].rearrange("p (n c) -> p c n", c=COUT),
                axis=mybir.AxisListType.X, op=Alu.add)
            r2 = cpool.tile([P, COUT], dt.float32)
            scr = cpool.tile([P, 512], dt.float32)
            r2c = cpool.tile([P, COUT], dt.float32)
            for ci, c0 in enumerate(range(0, ycols, 512)):
                c1 = min(c0 + 512, ycols)
                nc.scalar.activation(
                    out=scr[:, :c1 - c0], in_=y_sb[:, c0:c1],
                    func=Act.Square)
                nc.vector.tensor_reduce(
                    out=(r2 if ci == 0 else r2c)[:],
                    in_=scr[:, :c1 - c0].rearrange(
                        "p (n c) -> p c n", c=COUT),
                    axis=mybir.AxisListType.X, op=Alu.add)
                if ci > 0:
                    nc.vector.tensor_tensor(
                        out=r2[:], in0=r2[:], in1=r2c[:], op=Alu.add)
            r12 = cpool.tile([P, 2 * COUT], dt.float32)
            nc.vector.tensor_copy(out=r12[:, :COUT], in_=r1[:])
            nc.vector.tensor_copy(out=r12[:, COUT:], in_=r2[:])
            sp = sppool.tile([1, 2 * COUT], dt.float32, tag="sp")
            nc.tensor.matmul(out=sp[:], lhsT=ones_c[:], rhs=r12[:],
                             start=True, stop=True)
            s12 = cpool.tile([1, 2 * COUT], dt.float32)
            nc.scalar.copy(out=s12[:], in_=sp[:])

            cc_in = dpool.tile([1, 2 * COUT], dt.float32)
            cc_out = dpool.tile([1, 2 * COUT], dt.float32)
            nc.sync.dma_start(out=cc_in[:], in_=s12[:])
            nc.gpsimd.collective_compute(
                "AllReduce", Alu.add,
                replica_groups=[list(range(NCORES))],
                ins=[cc_in.opt()], outs=[cc_out.opt()])
            s12r = cpool.tile([1, 2 * COUT], dt.float32)
            nc.sync.dma_start(out=s12r[:], in_=cc_out[:])

            mv = cpool.tile([1, 2 * COUT], dt.float32)
            nc.vector.tensor_scalar_mul(mv[:], s12r[:], 1.0 / n_total)
            mean2 = cpool.tile([1, COUT], dt.float32)
            nc.vector.tensor_tensor(
                out=mean2[:], in0=mv[:, :COUT], in1=mv[:, :COUT],
                op=Alu.mult)
            var = cpool.tile([1, COUT], dt.float32)
            nc.vector.tensor_tensor(
                out=var[:], in0=mv[:, COUT:], in1=mean2[:], op=Alu.subtract)
            nc.vector.tensor_scalar_add(var[:], var[:], EPS)
            std = cpool.tile([1, COUT], dt.float32)
            nc.scalar.activation(out=std[:], in_=var[:], func=Act.Sqrt)
            rstd = cpool.tile([1, COUT], dt.float32)
            nc.vector.reciprocal(out=rstd[:], in_=std[:])
            sc_row = cpool.tile([1, COUT], dt.float32)
            nc.vector.tensor_tensor(
                out=sc_row[:], in0=gam[:], in1=rstd[:], op=Alu.mult)
            sh_row = cpool.tile([1, COUT], dt.float32)
            nc.vector.tensor_tensor(
                out=sh_row[:], in0=mv[:, :COUT], in1=sc_row[:], op=Alu.mult)
            nc.vector.tensor_tensor(
                out=sh_row[:], in0=bet[:], in1=sh_row[:], op=Alu.subtract)

            # broadcast rows -> [P, 32] via outer product, then tile to 512
            ssp = sppool.tile([P, 2 * COUT], dt.float32, tag="sp")
            nc.tensor.matmul(out=ssp[:, :COUT], lhsT=ones_r[:],
                             rhs=sc_row[:], start=True, stop=True)
            nc.tensor.matmul(out=ssp[:, COUT:], lhsT=ones_r[:],
                             rhs=sh_row[:], start=True, stop=True)
            sc_rep = cpool.tile([P, 512], dt.float32)
            sh_rep = cpool.tile([P, 512], dt.float32)
            for r in range(512 // COUT):
                nc.scalar.copy(
                    out=sc_rep[:, r * COUT:(r + 1) * COUT], in_=ssp[:, :COUT])
                nc.scalar.copy(
                    out=sh_rep[:, r * COUT:(r + 1) * COUT], in_=ssp[:, COUT:])

            # ---- pass 2: y = relu(y*scale + shift), store ----
            for c0 in range(0, ycols, 512):
                c1 = min(c0 + 512, ycols)
                w = c1 - c0
                nc.vector.tensor_tensor(
                    out=y_sb[:, c0:c1], in0=y_sb[:, c0:c1],
                    in1=sc_rep[:, :w], op=Alu.mult)
                nc.vector.tensor_tensor(
                    out=y_sb[:, c0:c1], in0=y_sb[:, c0:c1],
                    in1=sh_rep[:, :w], op=Alu.add)
                nc.vector.tensor_scalar_max(y_sb[:, c0:c1], y_sb[:, c0:c1],
                                            0.0)
            for c0 in range(0, ycols, 4096):
                c1 = min(c0 + 4096, ycols)
                nc.sync.dma_start(out=out_d[:, c0:c1], in_=y_sb[:, c0:c1])
    return nc


def _install_ntff_hook():
    if "antenv.axon_hooks" in sys.modules:
        return
    try:
        lib = ctypes.CDLL("/opt/axon/libaxon_pjrt.so")
        lib.axon_start_nrt_profile.argtypes = [
            ctypes.POINTER(ctypes.c_int64), ctypes.c_size_t]
        lib.axon_start_nrt_profile.restype = ctypes.c_int64
        lib.axon_stop_nrt_profile.argtypes = [ctypes.c_char_p]
        lib.axon_stop_nrt_profile.restype = ctypes.c_int64
    except OSError:
        return

    @contextlib.contextmanager
    def _hook(output_dir, device_ids):
        import jax
        jax.devices()
        if device_ids:
            ids = (ctypes.c_int64 * len(device_ids))(*device_ids)
            rc = lib.axon_start_nrt_profile(ids, len(device_ids))
        else:
            rc = lib.axon_start_nrt_profile(None, 0)
        if rc != 0:
            raise RuntimeError(f"axon_start_nrt_profile rc={rc}")
        try:
            yield
        finally:
            n = lib.axon_stop_nrt_profile(str(output_dir).encode())
            if n <= 0:
                print(f"profile: {n} files in {output_dir}", file=sys.stderr)

    mod = types.ModuleType("antenv.axon_hooks")
    mod.get_axon_ntff_profile_hook = lambda: _hook
    mod.set_axon_ntff_profile_hook = lambda h: None
    sys.modules["antenv.axon_hooks"] = mod


_NC_CACHE = {}


def _get_nc(schedule):
    key = tuple(schedule)
    if key not in _NC_CACHE:
        nc = _build(schedule, NROWS)
        nc.finalize()
        _NC_CACHE[key] = nc
    return _NC_CACHE[key]


def kernel(feats, W, gamma, beta, nbr_idx, nbr_mask, trace=False):
    feats = np.asarray(feats, np.float32)
    W = np.asarray(W, np.float32)
    gamma = np.asarray(gamma, np.float32)
    beta = np.asarray(beta, np.float32)
    nbr_idx = np.asarray(nbr_idx, np.int32)
    nbr_mask = np.asarray(nbr_mask, bool)
    n = feats.shape[0]
    assert n == NROWS and n % NCORES == 0
    per_core = n // NCORES

    feats_b = np.zeros((NJROWS, CIN), ml_dtypes.bfloat16)
    feats_b[:n] = feats.astype(ml_dtypes.bfloat16)
    # block-diagonal weight bands for tile-pair (64-contraction) matmuls:
    # rows [64p .. 64p+32) carry W at cols 0..863, rows [64p+32 .. 64p+64)
    # carry W at cols 864..1727
    wall = np.ascontiguousarray(
        W.transpose(1, 0, 2).reshape(CIN, K * COUT))
    wall8 = np.zeros((P, 2 * K * COUT), np.float32)
    for p_ in range(2):
        wall8[64 * p_:64 * p_ + 32, :K * COUT] = wall
        wall8[64 * p_ + 32:64 * p_ + 64, K * COUT:] = wall
    wall8 = wall8.astype(ml_dtypes.bfloat16)
    gamma_r = gamma.reshape(1, COUT)
    beta_r = beta.reshape(1, COUT)
    ones_r = np.ones((1, P), np.float32)

    # packed jk indices: valid neighbors first, ZROW padding
    jk = nbr_idx * np.int32(K) + np.arange(K, dtype=np.int32)[None, :]
    ordk = np.argsort(~nbr_mask, axis=1, kind="stable")
    jkp = np.take_along_axis(jk, ordk, 1)
    mskp = np.take_along_axis(nbr_mask, ordk, 1)
    jkp = np.where(mskp, jkp, np.int32(ZROW))
    counts = nbr_mask.sum(1).astype(np.int32)

    orders = []
    cts = np.zeros((NCORES, T_TILES), np.int32)
    for c in range(NCORES):
        lo = c * per_core
        cs = counts[lo:lo + per_core]
        order = np.argsort(-cs, kind="stable")
        orders.append(order)
        sc = np.concatenate(
            [cs[order], np.zeros(T_TILES * P - per_core, np.int32)])
        cts[c] = sc[::P][:T_TILES]
    schedule = cts.max(0)
    totc = int(schedule.sum())
    col_base = np.concatenate([[0], np.cumsum(schedule)[:-1]])

    in_maps = []
    for c in range(NCORES):
        lo = c * per_core
        R = np.full((T_TILES * P, K), ZROW, np.int32)
        R[:per_core] = jkp[lo:lo + per_core][orders[c]]
        R3 = R.reshape(T_TILES, P, K)
        pk = np.full((P, totc), ZROW, np.int32)
        for t in range(T_TILES):
            s = int(schedule[t])
            if s:
                pk[:, col_base[t]:col_base[t] + s] = R3[t][:, :s]
        in_maps.append(dict(
            feats_b=feats_b, wall8=wall8, pk=pk,
            gamma_r=gamma_r, beta_r=beta_r, ones_r=ones_r))

    _install_ntff_hook()
    from concourse import bass_utils
    bass_utils.upload_artifacts = lambda tmpdir: tmpdir
    nc = _get_nc(schedule.tolist())
    res = bass_utils.run_bass_kernel_spmd(
        nc, in_maps, core_ids=list(range(NCORES)), trace=trace)

    chunks = []
    for c in range(NCORES):
        o = res.results[c]["out_sh"].reshape(P, T_TILES, COUT)
        ys = o.transpose(1, 0, 2).reshape(T_TILES * P, COUT)
        rc = np.empty((per_core, COUT), np.float32)
        rc[orders[c]] = ys[:per_core]
        chunks.append(rc)
    out = np.concatenate(chunks, axis=0)
    if trace:
        kernel.last_exec_time_ns = res.exec_time_ns
        kernel.last_trace = (res.instructions_and_trace or (None, None))[1]
    return out
